# revision 35
# baseline (speedup 1.0000x reference)
"""Bahdanau attention (with coverage) Trainium2 Bass kernel.

Computes, for h_i (B,T,D), s_t (B,D), coverage (B,T) and projection
weights W_h (D,D), W_s (D,D), b_s (D,), W_c (D,), V_w (D,):

    enc  = h_i @ W_h                       (B,T,D)
    dec  = s_t @ W_s + b_s                 (B,1,D)
    covf = coverage[..., None] * W_c       (B,T,D)
    e_t  = tanh(enc + dec + covf) @ V_w    (B,T)
    a_t  = softmax(e_t, axis=1)
    coverage_new = coverage + a_t
    context = einsum('bt,btd->bd', a_t, h_i)

Sharding: data-parallel over batch, 4 examples per NeuronCore on 8 cores,
weights replicated, no cross-core communication.

Implementation notes:
  - All heavy matmuls run in bf16 (1 cycle/row on the PE; fp32r measures
    2 cycles/row on TRN2 silicon) accumulating in fp32 PSUM.
  - The enc matmul contracts over d, so it consumes h^T tiles (d on
    partitions); h^T is prepared host-side (pure layout prep).
  - FT tiles are (e-partitions, t-free); the coverage rank-1 term
    (Wc outer cov) is added by the DVE straight into the FT PSUM tile
    (the coverage row is broadcast across partitions once per batch by a
    ones outer-product on the PE during warm-up); dec enters as the
    per-partition bias of the ScalarE tanh (dec itself is 0.1% of the
    FLOPs, computed host-side).
  - e_t = V_w . tanh(FT) accumulates via M=1 matmuls, software-pipelined
    two m-steps behind the FT matmuls so the PE never waits on the
    DVE/ScalarE consumers.
  - softmax skips max-subtraction (logits are O(30) at most); exp + sum
    happen in one ScalarE pass via accum_out; a_t and coverage_new are
    produced in fp32.
  - context uses natural-layout h with the PE-transposed unnormalized exp
    vector as stationary operand; 1/sum is folded into the final copy.
    The context/transpose PE work of batch b is emitted inside batch
    b+1's stream so the PE tail stays busy.
  - all bulk DMAs ride one sync-queue FIFO in deadline order: W_h and
    batch 0's h^T first (k-tile granular), then each batch's next-hT /
    natural-h loads are issued from fixed points inside the m-loop.
    Zero-data warm-up matmuls cover the initial DMA window and lift the
    PE clock (HAM) to full rate before the real stream starts.
"""

import numpy as np

B, T, D = 32, 1024, 1024
NCORES = 8
BPC = B // NCORES  # batches per core
P = 128            # partitions
KT = D // P        # 8 contraction tiles
MT = D // P        # 8 output (e) tiles
NH = 2             # halves of the t/free dim (512 each)
NHS = 512
WARM_MMS = 26      # zero-data PE warm-up matmuls at kernel start

_cache = {}


def _build_nc():
    import concourse.tile as tile
    from concourse import bacc, mybir
    from concourse.masks import make_identity

    F32 = mybir.dt.float32
    BF16 = mybir.dt.bfloat16
    AF = mybir.ActivationFunctionType

    nc = bacc.Bacc("TRN2", target_bir_lowering=False, debug=False,
                   num_devices=NCORES)

    hT_d = nc.dram_tensor("hT", [BPC, D, T], BF16, kind="ExternalInput")
    h_d = nc.dram_tensor("h", [BPC, T, D], BF16, kind="ExternalInput")
    Wh_d = nc.dram_tensor("Wh", [D, D], BF16, kind="ExternalInput")
    decT_d = nc.dram_tensor("decT", [D, BPC], F32, kind="ExternalInput")
    cov_d = nc.dram_tensor("cov", [BPC, T], F32, kind="ExternalInput")
    covb_d = nc.dram_tensor("covb", [BPC, T], BF16, kind="ExternalInput")
    Wc_d = nc.dram_tensor("Wc", [D], BF16, kind="ExternalInput")
    Vw_d = nc.dram_tensor("Vw", [D], BF16, kind="ExternalInput")

    ctx_d = nc.dram_tensor("ctx", [BPC, D], F32, kind="ExternalOutput")
    a_d = nc.dram_tensor("a", [BPC, T], F32, kind="ExternalOutput")
    covn_d = nc.dram_tensor("covn", [BPC, T], F32, kind="ExternalOutput")

    with tile.TileContext(nc) as tc:
        with (
            tc.tile_pool(name="const", bufs=1) as cpool,
            tc.tile_pool(name="hTk", bufs=2) as hTpool,
            tc.tile_pool(name="hnat", bufs=2) as hnpool,
            tc.tile_pool(name="tanhp", bufs=3) as tpool,
            tc.tile_pool(name="rows", bufs=2) as rpool,
        ):
            # --- front-loaded DMAs: Wh + batch0 hT on the sync queue ---
            Wh_s = cpool.tile([P, KT, D], BF16)
            hT0 = [hTpool.tile([P, T], BF16, tag=f"hT{kt}",
                               name=f"hT0_{kt}")
                   for kt in range(KT)]
            for kt in range(KT):
                nc.sync.dma_start(
                    Wh_s[:, kt, 0:P],
                    Wh_d.ap()[kt * P:(kt + 1) * P, 0:P])
                nc.sync.dma_start(hT0[kt][:], hT_d.ap()[0, kt * P:(kt + 1) * P, :])
            for kt in range(KT):
                nc.sync.dma_start(
                    Wh_s[:, kt, P:D],
                    Wh_d.ap()[kt * P:(kt + 1) * P, P:D])

            # --- small constants on the gpsimd queue ---
            zero_bf = cpool.tile([P, NHS], BF16)
            nc.vector.memset(zero_bf[:], 0.0)
            V_col = cpool.tile([P, MT], BF16)
            nc.gpsimd.dma_start(
                V_col[:], Vw_d.ap().rearrange("(m p) -> p m", p=P))
            ones_bf = cpool.tile([1, P], BF16)
            nc.vector.memset(ones_bf[:], 1.0)
            Wc_col = cpool.tile([P, MT], F32)
            nc.gpsimd.dma_start(
                Wc_col[:], Wc_d.ap().rearrange("(m p) -> p m", p=P))
            dec_T = cpool.tile([P, MT, BPC], F32)
            nc.gpsimd.dma_start(
                dec_T[:], decT_d.ap().rearrange("(m p) b -> p m b", p=P))
            ident = cpool.tile([P, P], F32)
            make_identity(nc, ident[:])

            covb_rows = cpool.tile([1, BPC, T], BF16)
            nc.gpsimd.dma_start(covb_rows[:], covb_d.ap()[None, :, :])
            covbc_all = cpool.tile([P, BPC, T], BF16)

            # ---- PE warm-up on a zero tile: keeps the PE busy during
            # the initial weight/hT DMAs and lifts HAM to 8/8 before the
            # real matmul stream starts. The coverage rows are broadcast
            # across partitions here too (ones outer-product on the PE).
            with tc.tile_pool(name="warm", bufs=2, space="PSUM") as warmpool:
                warm_ps = warmpool.tile([P, NHS], F32, name="warmz")
                for i in range(WARM_MMS):
                    nc.tensor.matmul(
                        warm_ps[:], zero_bf[:, 0:P], zero_bf[:],
                        start=True, stop=True)
                for b in range(BPC):
                    for n in range(NH):
                        bc = warmpool.tile([P, NHS], F32, tag="bc",
                                           name=f"bc{b}_{n}")
                        nc.tensor.matmul(
                            bc[:], ones_bf[:],
                            covb_rows[0:1, b, n * NHS:(n + 1) * NHS],
                            start=True, stop=True)
                        nc.vector.tensor_copy(
                            covbc_all[:, b, n * NHS:(n + 1) * NHS], bc[:])

            # ---- main per-batch pipeline ----
            with (
                tc.tile_pool(name="ft_ps", bufs=2, space="PSUM") as ftpool,
                tc.tile_pool(name="et_ps", bufs=1, space="PSUM") as etpool,
                tc.tile_pool(name="tr_ps", bufs=1, space="PSUM") as trpool,
                tc.tile_pool(name="ctx_ps", bufs=1, space="PSUM") as cxpool,
            ):
                state = {}

                def emit_tail(b, hs, exp_row, rsum):
                    """PE part of batch b's epilogue (emitted inside b+1)."""
                    expT_ps = trpool.tile([P, KT], F32, tag="expT_ps")
                    for kt in range(KT):
                        nc.tensor.transpose(
                            expT_ps[:, kt:kt + 1],
                            exp_row[0:1, kt * P:(kt + 1) * P],
                            ident[0:1, 0:1])
                    expT = rpool.tile([P, KT], BF16, tag="expT")
                    nc.vector.tensor_copy(expT[:], expT_ps[:])

                    ctx_row = rpool.tile([1, D], F32, tag="ctx")
                    for n in range(NH):
                        ctx_ps = cxpool.tile([1, NHS], F32, tag="ctxps")
                        for kt in range(KT):
                            nc.tensor.matmul(
                                ctx_ps[:],
                                expT[:, kt:kt + 1],
                                hs[:, kt, n * NHS:(n + 1) * NHS],
                                start=(kt == 0), stop=(kt == KT - 1))
                        nc.scalar.activation(
                            ctx_row[0:1, n * NHS:(n + 1) * NHS], ctx_ps[:],
                            AF.Copy, scale=rsum[:])
                    nc.sync.dma_start(ctx_d.ap()[b:b + 1, :], ctx_row[:])

                def load_hT(b):
                    hTs = [hTpool.tile([P, T], BF16, tag=f"hT{kt}",
                                       name=f"hT{b}_{kt}")
                           for kt in range(KT)]
                    for kt in range(KT):
                        nc.sync.dma_start(
                            hTs[kt][:], hT_d.ap()[b, kt * P:(kt + 1) * P, :])
                    return hTs

                def load_h(b):
                    h_s = hnpool.tile([P, KT, D], BF16, tag="h",
                                      name=f"h_s{b}")
                    nc.sync.dma_start(
                        h_s[:],
                        h_d.ap()[b].rearrange("(kt p) d -> p kt d", p=P))
                    return h_s

                next_hTs = None
                for b in range(BPC):
                    hTs = hT0 if b == 0 else next_hTs
                    h_s = None  # loaded mid-loop (deadline: our tail)
                    cov_row = rpool.tile([1, T], F32, tag="cov")
                    nc.gpsimd.dma_start(cov_row[:], cov_d.ap()[b:b + 1, :])

                    # rank-1 coverage tiles: build two ahead of the m-loop,
                    # then one per iteration, so the DVE never bursts and
                    # delays the ft adds
                    covWc = {}

                    def build_covWc(m):
                        cw = tpool.tile([P, T], BF16, tag=f"covWc{m}",
                                        bufs=2, name=f"covWc{b}_{m}")
                        nc.vector.tensor_scalar_mul(
                            cw[:], covbc_all[:, b, :], Wc_col[:, m:m + 1])
                        covWc[m] = cw

                    build_covWc(0)
                    build_covWc(1)
                    et_ps = etpool.tile([1, T], F32)
                    tanh_tiles = {}
                    for m in range(MT):
                        ft = ftpool.tile([P, T], F32, tag="ft",
                                         name=f"ft{b}_{m}")
                        if b == 0 and 1 <= m <= 4:
                            # keep the PE warm through batch-0's DMA dribble
                            for _ in range(6):
                                nc.tensor.matmul(
                                    ft[:, 0:NHS], zero_bf[:, 0:P],
                                    zero_bf[:], start=True, stop=True)
                        for kt in range(KT):
                            for n in range(NH):
                                nc.tensor.matmul(
                                    ft[:, n * NHS:(n + 1) * NHS],
                                    Wh_s[:, kt, m * P:(m + 1) * P],
                                    hTs[kt][:, n * NHS:(n + 1) * NHS],
                                    start=(kt == 0), stop=(kt == KT - 1))
                        if m + 2 < MT:
                            build_covWc(m + 2)
                        nc.vector.tensor_add(ft[:], ft[:], covWc.pop(m)[:])
                        tanh_t = tpool.tile([P, T], BF16, tag="tanh")
                        nc.scalar.activation(
                            tanh_t[:], ft[:], AF.Tanh,
                            bias=dec_T[:, m, b:b + 1])
                        tanh_tiles[m] = tanh_t

                        # previous batch's PE tail after our first m-tile
                        if m == 1 and b > 0:
                            emit_tail(b - 1, *state.pop(b - 1))
                        # DMA emission order = deadline order on one queue:
                        # next batch's hT before our h (used one tail later)
                        if m == 2 and b < BPC - 1:
                            next_hTs = load_hT(b + 1)
                        if m == 5:
                            h_s = load_h(b)

                        # e_t matmuls, two m-steps behind the FT matmuls
                        # (hides the DVE rank-1 add + tanh latency)
                        if m > 1:
                            prev = tanh_tiles.pop(m - 2)
                            for n in range(NH):
                                nc.tensor.matmul(
                                    et_ps[0:1, n * NHS:(n + 1) * NHS],
                                    V_col[:, m - 2:m - 1],
                                    prev[:, n * NHS:(n + 1) * NHS],
                                    start=(m == 2), stop=False)
                    for mm in (MT - 2, MT - 1):
                        last = tanh_tiles.pop(mm)
                        for n in range(NH):
                            nc.tensor.matmul(
                                et_ps[0:1, n * NHS:(n + 1) * NHS],
                                V_col[:, mm:mm + 1],
                                last[:, n * NHS:(n + 1) * NHS],
                                start=False, stop=(mm == MT - 1))

                    # softmax pieces on ScalarE/DVE (no PE dependency)
                    exp_row = rpool.tile([1, T], F32, tag="exp")
                    esum = rpool.tile([1, 1], F32, tag="esum")
                    nc.scalar.activation(
                        exp_row[:], et_ps[:], AF.Exp, accum_out=esum[:])
                    rsum = rpool.tile([1, 1], F32, tag="rsum")
                    nc.vector.reciprocal(rsum[:], esum[:])

                    a_row = rpool.tile([1, T], F32, tag="a")
                    nc.scalar.activation(
                        a_row[:], exp_row[:], AF.Copy, scale=rsum[:])
                    nc.sync.dma_start(a_d.ap()[b:b + 1, :], a_row[:])
                    covn_row = rpool.tile([1, T], F32, tag="covn")
                    nc.vector.tensor_add(
                        covn_row[:], a_row[:], cov_row[0:1, :])
                    nc.sync.dma_start(covn_d.ap()[b:b + 1, :], covn_row[:])

                    state[b] = (h_s, exp_row, rsum)

                emit_tail(BPC - 1, *state.pop(BPC - 1))

    nc.compile()
    return nc


def _get_nc():
    if "nc" not in _cache:
        _cache["nc"] = _build_nc()
    return _cache["nc"]


def prep_in_maps(h_i, s_t, coverage, W_h, W_s, b_s, W_c, V_w):
    import ml_dtypes

    bf16 = ml_dtypes.bfloat16
    h_i = np.ascontiguousarray(np.asarray(h_i, dtype=np.float32))
    s_t = np.asarray(s_t, dtype=np.float32)
    coverage = np.ascontiguousarray(np.asarray(coverage, dtype=np.float32))
    W_h = np.asarray(W_h, dtype=np.float32)
    W_s = np.asarray(W_s, dtype=np.float32)
    b_s = np.asarray(b_s, dtype=np.float32)
    W_c = np.asarray(W_c, dtype=np.float32)
    V_w = np.asarray(V_w, dtype=np.float32)

    h_bf = h_i.astype(bf16)
    hT_bf = np.ascontiguousarray(h_i.transpose(0, 2, 1)).astype(bf16)
    Wh_bf = np.ascontiguousarray(W_h.astype(bf16))
    covb = coverage.astype(bf16)
    Wc_bf = np.ascontiguousarray(W_c.astype(bf16))
    Vw_bf = np.ascontiguousarray(V_w.astype(bf16))
    dec = s_t @ W_s + b_s[None, :]  # (B, D) fp32, 0.1% of the FLOPs

    in_maps = []
    for c in range(NCORES):
        lo, hi = c * BPC, (c + 1) * BPC
        decT = np.ascontiguousarray(dec[lo:hi].T)
        in_maps.append({
            "hT": hT_bf[lo:hi],
            "h": h_bf[lo:hi],
            "Wh": Wh_bf,
            "decT": decT,
            "cov": coverage[lo:hi],
            "covb": covb[lo:hi],
            "Wc": Wc_bf,
            "Vw": Vw_bf,
        })
    return in_maps


def kernel(h_i, s_t, coverage, W_h, W_s, b_s, W_c, V_w):
    import time

    from concourse import bass_utils

    in_maps = prep_in_maps(h_i, s_t, coverage, W_h, W_s, b_s, W_c, V_w)
    nc = _get_nc()
    res = None
    for attempt in range(3):
        try:
            res = bass_utils.run_bass_kernel_spmd(
                nc, in_maps, core_ids=list(range(NCORES)))
            break
        except Exception:
            # transient NRT device errors recover on retry
            if attempt == 2:
                raise
            time.sleep(10)

    ctx = np.concatenate([res.results[c]["ctx"] for c in range(NCORES)], 0)
    a_t = np.concatenate([res.results[c]["a"] for c in range(NCORES)], 0)
    covn = np.concatenate([res.results[c]["covn"] for c in range(NCORES)], 0)
    return ctx, a_t, covn


# revision 36
# speedup vs baseline: 1.0132x; 1.0132x over previous
"""Bahdanau attention (with coverage) Trainium2 Bass kernel.

Computes, for h_i (B,T,D), s_t (B,D), coverage (B,T) and projection
weights W_h (D,D), W_s (D,D), b_s (D,), W_c (D,), V_w (D,):

    enc  = h_i @ W_h                       (B,T,D)
    dec  = s_t @ W_s + b_s                 (B,1,D)
    covf = coverage[..., None] * W_c       (B,T,D)
    e_t  = tanh(enc + dec + covf) @ V_w    (B,T)
    a_t  = softmax(e_t, axis=1)
    coverage_new = coverage + a_t
    context = einsum('bt,btd->bd', a_t, h_i)

Sharding: data-parallel over batch, 4 examples per NeuronCore on 8 cores,
weights replicated, no cross-core communication.

Implementation notes:
  - All heavy matmuls run in bf16 (1 cycle/row on the PE; fp32r measures
    2 cycles/row on TRN2 silicon) accumulating in fp32 PSUM.
  - The enc matmul contracts over d, so it consumes h^T tiles (d on
    partitions); h^T is prepared host-side (pure layout prep).
  - FT tiles are (e-partitions, t-free); the coverage rank-1 term
    (Wc outer cov) is added by the DVE straight into the FT PSUM tile
    (the coverage row is broadcast across partitions once per batch by a
    ones outer-product on the PE during warm-up); dec enters as the
    per-partition bias of the ScalarE tanh (dec itself is 0.1% of the
    FLOPs, computed host-side).
  - e_t = V_w . tanh(FT) accumulates via M=1 matmuls, software-pipelined
    two m-steps behind the FT matmuls so the PE never waits on the
    DVE/ScalarE consumers.
  - softmax skips max-subtraction (logits are O(30) at most); exp + sum
    happen in one ScalarE pass via accum_out; a_t and coverage_new are
    produced in fp32.
  - context uses natural-layout h with the PE-transposed unnormalized exp
    vector as stationary operand; 1/sum is folded into the final copy.
    The context/transpose PE work of batch b is emitted inside batch
    b+1's stream so the PE tail stays busy.
  - all bulk DMAs ride one sync-queue FIFO in deadline order: W_h and
    batch 0's h^T first (k-tile granular), then each batch's next-hT /
    natural-h loads are issued from fixed points inside the m-loop.
    Zero-data warm-up matmuls cover the initial DMA window and lift the
    PE clock (HAM) to full rate before the real stream starts.
"""

import numpy as np

B, T, D = 32, 1024, 1024
NCORES = 8
BPC = B // NCORES  # batches per core
P = 128            # partitions
KT = D // P        # 8 contraction tiles
MT = D // P        # 8 output (e) tiles
NH = 2             # halves of the t/free dim (512 each)
NHS = 512
WARM_MMS = 26      # zero-data PE warm-up matmuls at kernel start

_cache = {}


def _build_nc():
    import concourse.tile as tile
    from concourse import bacc, mybir
    from concourse.masks import make_identity

    F32 = mybir.dt.float32
    BF16 = mybir.dt.bfloat16
    AF = mybir.ActivationFunctionType

    nc = bacc.Bacc("TRN2", target_bir_lowering=False, debug=False,
                   num_devices=NCORES)

    hT_d = nc.dram_tensor("hT", [BPC, D, T], BF16, kind="ExternalInput")
    h_d = nc.dram_tensor("h", [BPC, T, D], BF16, kind="ExternalInput")
    Wh_d = nc.dram_tensor("Wh", [D, D], BF16, kind="ExternalInput")
    decT_d = nc.dram_tensor("decT", [D, BPC], F32, kind="ExternalInput")
    cov_d = nc.dram_tensor("cov", [BPC, T], F32, kind="ExternalInput")
    covb_d = nc.dram_tensor("covb", [BPC, T], BF16, kind="ExternalInput")
    Wc_d = nc.dram_tensor("Wc", [D], BF16, kind="ExternalInput")
    Vw_d = nc.dram_tensor("Vw", [D], BF16, kind="ExternalInput")

    ctx_d = nc.dram_tensor("ctx", [BPC, D], F32, kind="ExternalOutput")
    a_d = nc.dram_tensor("a", [BPC, T], F32, kind="ExternalOutput")
    covn_d = nc.dram_tensor("covn", [BPC, T], F32, kind="ExternalOutput")

    with tile.TileContext(nc) as tc:
        with (
            tc.tile_pool(name="const", bufs=1) as cpool,
            tc.tile_pool(name="hTk", bufs=2) as hTpool,
            tc.tile_pool(name="hnat", bufs=2) as hnpool,
            tc.tile_pool(name="tanhp", bufs=3) as tpool,
            tc.tile_pool(name="rows", bufs=2) as rpool,
        ):
            # --- front-loaded DMAs: Wh + batch0 hT on the sync queue ---
            Wh_s = cpool.tile([P, KT, D], BF16)
            hT0 = [hTpool.tile([P, T], BF16, tag=f"hT{kt}",
                               name=f"hT0_{kt}")
                   for kt in range(KT)]
            for kt in range(KT):
                nc.sync.dma_start(
                    Wh_s[:, kt, 0:P],
                    Wh_d.ap()[kt * P:(kt + 1) * P, 0:P])
                nc.sync.dma_start(hT0[kt][:], hT_d.ap()[0, kt * P:(kt + 1) * P, :])
            for kt in range(KT):
                nc.sync.dma_start(
                    Wh_s[:, kt, P:D],
                    Wh_d.ap()[kt * P:(kt + 1) * P, P:D])

            # --- small constants on the gpsimd queue ---
            zero_bf = cpool.tile([P, NHS], BF16)
            nc.vector.memset(zero_bf[:], 0.0)
            V_col = cpool.tile([P, MT], BF16)
            nc.gpsimd.dma_start(
                V_col[:], Vw_d.ap().rearrange("(m p) -> p m", p=P))
            ones_bf = cpool.tile([1, P], BF16)
            nc.vector.memset(ones_bf[:], 1.0)
            Wc_col = cpool.tile([P, MT], F32)
            nc.gpsimd.dma_start(
                Wc_col[:], Wc_d.ap().rearrange("(m p) -> p m", p=P))
            dec_T = cpool.tile([P, MT, BPC], F32)
            nc.gpsimd.dma_start(
                dec_T[:], decT_d.ap().rearrange("(m p) b -> p m b", p=P))
            ident = cpool.tile([P, P], F32)
            make_identity(nc, ident[:])

            covb_rows = cpool.tile([1, BPC, T], BF16)
            nc.gpsimd.dma_start(covb_rows[:], covb_d.ap()[None, :, :])
            covbc_all = cpool.tile([P, BPC, T], BF16)

            # ---- PE warm-up on a zero tile: keeps the PE busy during
            # the initial weight/hT DMAs and lifts HAM to 8/8 before the
            # real matmul stream starts. The coverage rows are broadcast
            # across partitions here too (ones outer-product on the PE).
            with tc.tile_pool(name="warm", bufs=2, space="PSUM") as warmpool:
                warm_ps = warmpool.tile([P, NHS], F32, name="warmz")
                for i in range(WARM_MMS):
                    nc.tensor.matmul(
                        warm_ps[:], zero_bf[:, 0:P], zero_bf[:],
                        start=True, stop=True)
                for b in range(BPC):
                    for n in range(NH):
                        bc = warmpool.tile([P, NHS], F32, tag="bc",
                                           name=f"bc{b}_{n}")
                        nc.tensor.matmul(
                            bc[:], ones_bf[:],
                            covb_rows[0:1, b, n * NHS:(n + 1) * NHS],
                            start=True, stop=True)
                        nc.vector.tensor_copy(
                            covbc_all[:, b, n * NHS:(n + 1) * NHS], bc[:])

            # ---- main per-batch pipeline ----
            with (
                tc.tile_pool(name="ft_ps", bufs=2, space="PSUM") as ftpool,
                tc.tile_pool(name="et_ps", bufs=1, space="PSUM") as etpool,
                tc.tile_pool(name="tr_ps", bufs=1, space="PSUM") as trpool,
                tc.tile_pool(name="ctx_ps", bufs=1, space="PSUM") as cxpool,
            ):
                state = {}

                def emit_tail(b, hs, exp_row, rsum):
                    """PE part of batch b's epilogue (emitted inside b+1)."""
                    expT_ps = trpool.tile([P, KT], F32, tag="expT_ps")
                    for kt in range(KT):
                        nc.tensor.transpose(
                            expT_ps[:, kt:kt + 1],
                            exp_row[0:1, kt * P:(kt + 1) * P],
                            ident[0:1, 0:1])
                    expT = rpool.tile([P, KT], BF16, tag="expT")
                    nc.vector.tensor_copy(expT[:], expT_ps[:])

                    ctx_row = rpool.tile([1, D], F32, tag="ctx")
                    for n in range(NH):
                        ctx_ps = cxpool.tile([1, NHS], F32, tag="ctxps")
                        for kt in range(KT):
                            nc.tensor.matmul(
                                ctx_ps[:],
                                expT[:, kt:kt + 1],
                                hs[:, kt, n * NHS:(n + 1) * NHS],
                                start=(kt == 0), stop=(kt == KT - 1))
                        nc.scalar.activation(
                            ctx_row[0:1, n * NHS:(n + 1) * NHS], ctx_ps[:],
                            AF.Copy, scale=rsum[:])
                    nc.sync.dma_start(ctx_d.ap()[b:b + 1, :], ctx_row[:])

                def load_hT(b):
                    hTs = [hTpool.tile([P, T], BF16, tag=f"hT{kt}",
                                       name=f"hT{b}_{kt}")
                           for kt in range(KT)]
                    for kt in range(KT):
                        nc.sync.dma_start(
                            hTs[kt][:], hT_d.ap()[b, kt * P:(kt + 1) * P, :])
                    return hTs

                def load_h(b):
                    h_s = hnpool.tile([P, KT, D], BF16, tag="h",
                                      name=f"h_s{b}")
                    nc.sync.dma_start(
                        h_s[:],
                        h_d.ap()[b].rearrange("(kt p) d -> p kt d", p=P))
                    return h_s

                next_hTs = None
                for b in range(BPC):
                    hTs = hT0 if b == 0 else next_hTs
                    h_s = None  # loaded mid-loop (deadline: our tail)
                    cov_row = rpool.tile([1, T], F32, tag="cov")
                    nc.gpsimd.dma_start(cov_row[:], cov_d.ap()[b:b + 1, :])

                    # rank-1 coverage tiles: build two ahead of the m-loop,
                    # then one per iteration, so the DVE never bursts and
                    # delays the ft adds
                    covWc = {}

                    def build_covWc(m):
                        cw = tpool.tile([P, T], BF16, tag=f"covWc{m}",
                                        bufs=2, name=f"covWc{b}_{m}")
                        nc.vector.tensor_scalar_mul(
                            cw[:], covbc_all[:, b, :], Wc_col[:, m:m + 1])
                        covWc[m] = cw

                    build_covWc(0)
                    build_covWc(1)
                    et_ps = etpool.tile([1, T], F32)
                    tanh_tiles = {}
                    for m in range(MT):
                        ft = ftpool.tile([P, T], F32, tag="ft",
                                         name=f"ft{b}_{m}")
                        if b == 0 and 1 <= m <= 2:
                            # keep the PE warm through batch-0's DMA dribble
                            for _ in range(4):
                                nc.tensor.matmul(
                                    ft[:, 0:NHS], zero_bf[:, 0:P],
                                    zero_bf[:], start=True, stop=True)
                        for kt in range(KT):
                            for n in range(NH):
                                nc.tensor.matmul(
                                    ft[:, n * NHS:(n + 1) * NHS],
                                    Wh_s[:, kt, m * P:(m + 1) * P],
                                    hTs[kt][:, n * NHS:(n + 1) * NHS],
                                    start=(kt == 0), stop=(kt == KT - 1))
                        if m + 2 < MT:
                            build_covWc(m + 2)
                        nc.vector.tensor_add(ft[:], ft[:], covWc.pop(m)[:])
                        tanh_t = tpool.tile([P, T], BF16, tag="tanh")
                        nc.scalar.activation(
                            tanh_t[:], ft[:], AF.Tanh,
                            bias=dec_T[:, m, b:b + 1])
                        tanh_tiles[m] = tanh_t

                        # previous batch's PE tail after our first m-tile
                        if m == 1 and b > 0:
                            emit_tail(b - 1, *state.pop(b - 1))
                        # DMA emission order = deadline order on one queue:
                        # next batch's hT before our h (used one tail later)
                        if m == 2 and b < BPC - 1:
                            next_hTs = load_hT(b + 1)
                        if m == 5:
                            h_s = load_h(b)

                        # e_t matmuls, two m-steps behind the FT matmuls
                        # (hides the DVE rank-1 add + tanh latency)
                        if m > 1:
                            prev = tanh_tiles.pop(m - 2)
                            for n in range(NH):
                                nc.tensor.matmul(
                                    et_ps[0:1, n * NHS:(n + 1) * NHS],
                                    V_col[:, m - 2:m - 1],
                                    prev[:, n * NHS:(n + 1) * NHS],
                                    start=(m == 2), stop=False)
                    for mm in (MT - 2, MT - 1):
                        last = tanh_tiles.pop(mm)
                        for n in range(NH):
                            nc.tensor.matmul(
                                et_ps[0:1, n * NHS:(n + 1) * NHS],
                                V_col[:, mm:mm + 1],
                                last[:, n * NHS:(n + 1) * NHS],
                                start=False, stop=(mm == MT - 1))

                    # softmax pieces on ScalarE/DVE (no PE dependency)
                    exp_row = rpool.tile([1, T], F32, tag="exp")
                    esum = rpool.tile([1, 1], F32, tag="esum")
                    nc.scalar.activation(
                        exp_row[:], et_ps[:], AF.Exp, accum_out=esum[:])
                    rsum = rpool.tile([1, 1], F32, tag="rsum")
                    nc.vector.reciprocal(rsum[:], esum[:])

                    a_row = rpool.tile([1, T], F32, tag="a")
                    nc.scalar.activation(
                        a_row[:], exp_row[:], AF.Copy, scale=rsum[:])
                    nc.sync.dma_start(a_d.ap()[b:b + 1, :], a_row[:])
                    covn_row = rpool.tile([1, T], F32, tag="covn")
                    nc.vector.tensor_add(
                        covn_row[:], a_row[:], cov_row[0:1, :])
                    nc.sync.dma_start(covn_d.ap()[b:b + 1, :], covn_row[:])

                    state[b] = (h_s, exp_row, rsum)

                emit_tail(BPC - 1, *state.pop(BPC - 1))

    nc.compile()
    return nc


def _get_nc():
    if "nc" not in _cache:
        _cache["nc"] = _build_nc()
    return _cache["nc"]


def prep_in_maps(h_i, s_t, coverage, W_h, W_s, b_s, W_c, V_w):
    import ml_dtypes

    bf16 = ml_dtypes.bfloat16
    h_i = np.ascontiguousarray(np.asarray(h_i, dtype=np.float32))
    s_t = np.asarray(s_t, dtype=np.float32)
    coverage = np.ascontiguousarray(np.asarray(coverage, dtype=np.float32))
    W_h = np.asarray(W_h, dtype=np.float32)
    W_s = np.asarray(W_s, dtype=np.float32)
    b_s = np.asarray(b_s, dtype=np.float32)
    W_c = np.asarray(W_c, dtype=np.float32)
    V_w = np.asarray(V_w, dtype=np.float32)

    h_bf = h_i.astype(bf16)
    hT_bf = np.ascontiguousarray(h_i.transpose(0, 2, 1)).astype(bf16)
    Wh_bf = np.ascontiguousarray(W_h.astype(bf16))
    covb = coverage.astype(bf16)
    Wc_bf = np.ascontiguousarray(W_c.astype(bf16))
    Vw_bf = np.ascontiguousarray(V_w.astype(bf16))
    dec = s_t @ W_s + b_s[None, :]  # (B, D) fp32, 0.1% of the FLOPs

    in_maps = []
    for c in range(NCORES):
        lo, hi = c * BPC, (c + 1) * BPC
        decT = np.ascontiguousarray(dec[lo:hi].T)
        in_maps.append({
            "hT": hT_bf[lo:hi],
            "h": h_bf[lo:hi],
            "Wh": Wh_bf,
            "decT": decT,
            "cov": coverage[lo:hi],
            "covb": covb[lo:hi],
            "Wc": Wc_bf,
            "Vw": Vw_bf,
        })
    return in_maps


def kernel(h_i, s_t, coverage, W_h, W_s, b_s, W_c, V_w):
    import time

    from concourse import bass_utils

    in_maps = prep_in_maps(h_i, s_t, coverage, W_h, W_s, b_s, W_c, V_w)
    nc = _get_nc()
    res = None
    for attempt in range(3):
        try:
            res = bass_utils.run_bass_kernel_spmd(
                nc, in_maps, core_ids=list(range(NCORES)))
            break
        except Exception:
            # transient NRT device errors recover on retry
            if attempt == 2:
                raise
            time.sleep(10)

    ctx = np.concatenate([res.results[c]["ctx"] for c in range(NCORES)], 0)
    a_t = np.concatenate([res.results[c]["a"] for c in range(NCORES)], 0)
    covn = np.concatenate([res.results[c]["covn"] for c in range(NCORES)], 0)
    return ctx, a_t, covn


# revision 37
# speedup vs baseline: 1.0175x; 1.0043x over previous
"""Bahdanau attention (with coverage) Trainium2 Bass kernel.

Computes, for h_i (B,T,D), s_t (B,D), coverage (B,T) and projection
weights W_h (D,D), W_s (D,D), b_s (D,), W_c (D,), V_w (D,):

    enc  = h_i @ W_h                       (B,T,D)
    dec  = s_t @ W_s + b_s                 (B,1,D)
    covf = coverage[..., None] * W_c       (B,T,D)
    e_t  = tanh(enc + dec + covf) @ V_w    (B,T)
    a_t  = softmax(e_t, axis=1)
    coverage_new = coverage + a_t
    context = einsum('bt,btd->bd', a_t, h_i)

Sharding: data-parallel over batch, 4 examples per NeuronCore on 8 cores,
weights replicated, no cross-core communication.

Implementation notes:
  - All heavy matmuls run in bf16 (1 cycle/row on the PE; fp32r measures
    2 cycles/row on TRN2 silicon) accumulating in fp32 PSUM.
  - The enc matmul contracts over d, so it consumes h^T tiles (d on
    partitions); h^T is prepared host-side (pure layout prep).
  - FT tiles are (e-partitions, t-free); the coverage rank-1 term
    (Wc outer cov) is added by the DVE straight into the FT PSUM tile
    (the coverage row is broadcast across partitions once per batch by a
    ones outer-product on the PE during warm-up); dec enters as the
    per-partition bias of the ScalarE tanh (dec itself is 0.1% of the
    FLOPs, computed host-side).
  - e_t = V_w . tanh(FT) accumulates via M=1 matmuls, software-pipelined
    two m-steps behind the FT matmuls so the PE never waits on the
    DVE/ScalarE consumers.
  - softmax skips max-subtraction (logits are O(30) at most); exp + sum
    happen in one ScalarE pass via accum_out; a_t and coverage_new are
    produced in fp32.
  - context uses natural-layout h with the PE-transposed unnormalized exp
    vector as stationary operand; 1/sum is folded into the final copy.
    The context/transpose PE work of batch b is emitted inside batch
    b+1's stream so the PE tail stays busy.
  - all bulk DMAs ride one sync-queue FIFO in deadline order: W_h and
    batch 0's h^T first (k-tile granular), then each batch's next-hT /
    natural-h loads are issued from fixed points inside the m-loop.
    Zero-data warm-up matmuls cover the initial DMA window and lift the
    PE clock (HAM) to full rate before the real stream starts.
"""

import numpy as np

B, T, D = 32, 1024, 1024
NCORES = 8
BPC = B // NCORES  # batches per core
P = 128            # partitions
KT = D // P        # 8 contraction tiles
MT = D // P        # 8 output (e) tiles
NH = 2             # halves of the t/free dim (512 each)
NHS = 512
WARM_MMS = 26      # zero-data PE warm-up matmuls at kernel start

_cache = {}


def _build_nc():
    import concourse.tile as tile
    from concourse import bacc, mybir
    from concourse.masks import make_identity

    F32 = mybir.dt.float32
    BF16 = mybir.dt.bfloat16
    AF = mybir.ActivationFunctionType

    nc = bacc.Bacc("TRN2", target_bir_lowering=False, debug=False,
                   num_devices=NCORES)

    hT_d = nc.dram_tensor("hT", [BPC, D, T], BF16, kind="ExternalInput")
    h_d = nc.dram_tensor("h", [BPC, T, D], BF16, kind="ExternalInput")
    Wh_d = nc.dram_tensor("Wh", [D, D], BF16, kind="ExternalInput")
    decT_d = nc.dram_tensor("decT", [D, BPC], F32, kind="ExternalInput")
    cov_d = nc.dram_tensor("cov", [BPC, T], F32, kind="ExternalInput")
    covb_d = nc.dram_tensor("covb", [BPC, T], BF16, kind="ExternalInput")
    Wc_d = nc.dram_tensor("Wc", [D], BF16, kind="ExternalInput")
    Vw_d = nc.dram_tensor("Vw", [D], BF16, kind="ExternalInput")

    ctx_d = nc.dram_tensor("ctx", [BPC, D], F32, kind="ExternalOutput")
    a_d = nc.dram_tensor("a", [BPC, T], F32, kind="ExternalOutput")
    covn_d = nc.dram_tensor("covn", [BPC, T], F32, kind="ExternalOutput")

    with tile.TileContext(nc) as tc:
        with (
            tc.tile_pool(name="const", bufs=1) as cpool,
            tc.tile_pool(name="hTk", bufs=2) as hTpool,
            tc.tile_pool(name="hnat", bufs=2) as hnpool,
            tc.tile_pool(name="tanhp", bufs=3) as tpool,
            tc.tile_pool(name="rows", bufs=2) as rpool,
        ):
            # --- front-loaded DMAs: Wh + batch0 hT on the sync queue ---
            Wh_s = cpool.tile([P, KT, D], BF16)
            hT0 = [hTpool.tile([P, T], BF16, tag=f"hT{kt}",
                               name=f"hT0_{kt}")
                   for kt in range(KT)]
            for kt in range(KT):
                nc.sync.dma_start(
                    Wh_s[:, kt, 0:P],
                    Wh_d.ap()[kt * P:(kt + 1) * P, 0:P])
                nc.sync.dma_start(hT0[kt][:], hT_d.ap()[0, kt * P:(kt + 1) * P, :])
            for kt in range(KT):
                nc.sync.dma_start(
                    Wh_s[:, kt, P:D],
                    Wh_d.ap()[kt * P:(kt + 1) * P, P:D])

            # --- small constants on the gpsimd queue ---
            zero_bf = cpool.tile([P, NHS], BF16)
            nc.vector.memset(zero_bf[:], 0.0)
            V_col = cpool.tile([P, MT], BF16)
            nc.gpsimd.dma_start(
                V_col[:], Vw_d.ap().rearrange("(m p) -> p m", p=P))
            ones_bf = cpool.tile([1, P], BF16)
            nc.vector.memset(ones_bf[:], 1.0)
            Wc_col = cpool.tile([P, MT], F32)
            nc.gpsimd.dma_start(
                Wc_col[:], Wc_d.ap().rearrange("(m p) -> p m", p=P))
            dec_T = cpool.tile([P, MT, BPC], F32)
            nc.gpsimd.dma_start(
                dec_T[:], decT_d.ap().rearrange("(m p) b -> p m b", p=P))
            ident = cpool.tile([P, P], F32)
            make_identity(nc, ident[:])

            covb_rows = cpool.tile([1, BPC, T], BF16)
            nc.gpsimd.dma_start(covb_rows[:], covb_d.ap()[None, :, :])
            covbc_all = cpool.tile([P, BPC, T], BF16)

            # ---- PE warm-up on a zero tile: keeps the PE busy during
            # the initial weight/hT DMAs and lifts HAM to 8/8 before the
            # real matmul stream starts. The coverage rows are broadcast
            # across partitions here too (ones outer-product on the PE).
            with tc.tile_pool(name="warm", bufs=2, space="PSUM") as warmpool:
                warm_ps = warmpool.tile([P, NHS], F32, name="warmz")
                for i in range(WARM_MMS):
                    nc.tensor.matmul(
                        warm_ps[:], zero_bf[:, 0:P], zero_bf[:],
                        start=True, stop=True)
                for b in range(BPC):
                    for n in range(NH):
                        bc = warmpool.tile([P, NHS], F32, tag="bc",
                                           name=f"bc{b}_{n}")
                        nc.tensor.matmul(
                            bc[:], ones_bf[:],
                            covb_rows[0:1, b, n * NHS:(n + 1) * NHS],
                            start=True, stop=True)
                        nc.vector.tensor_copy(
                            covbc_all[:, b, n * NHS:(n + 1) * NHS], bc[:])

            # ---- main per-batch pipeline ----
            with (
                tc.tile_pool(name="ft_ps", bufs=2, space="PSUM") as ftpool,
                tc.tile_pool(name="et_ps", bufs=1, space="PSUM") as etpool,
                tc.tile_pool(name="tr_ps", bufs=2, space="PSUM") as trpool,
            ):
                state = {}

                def emit_tail(b, hs, exp_row, rsum):
                    """PE part of batch b's epilogue (emitted inside b+1)."""
                    expT_ps = trpool.tile([P, KT], F32, tag="tr",
                                          name=f"expT_ps{b}")
                    for kt in range(KT):
                        nc.tensor.transpose(
                            expT_ps[:, kt:kt + 1],
                            exp_row[0:1, kt * P:(kt + 1) * P],
                            ident[0:1, 0:1])
                    expT = rpool.tile([P, KT], BF16, tag="expT")
                    nc.vector.tensor_copy(expT[:], expT_ps[:])

                    ctx_row = rpool.tile([1, D], F32, tag="ctx")
                    for n in range(NH):
                        ctx_ps = trpool.tile([1, NHS], F32, tag="tr",
                                             name=f"ctx_ps{b}_{n}")
                        for kt in range(KT):
                            nc.tensor.matmul(
                                ctx_ps[:],
                                expT[:, kt:kt + 1],
                                hs[:, kt, n * NHS:(n + 1) * NHS],
                                start=(kt == 0), stop=(kt == KT - 1))
                        nc.scalar.activation(
                            ctx_row[0:1, n * NHS:(n + 1) * NHS], ctx_ps[:],
                            AF.Copy, scale=rsum[:])
                    nc.sync.dma_start(ctx_d.ap()[b:b + 1, :], ctx_row[:])

                def load_hT(b):
                    hTs = [hTpool.tile([P, T], BF16, tag=f"hT{kt}",
                                       name=f"hT{b}_{kt}")
                           for kt in range(KT)]
                    for kt in range(KT):
                        nc.sync.dma_start(
                            hTs[kt][:], hT_d.ap()[b, kt * P:(kt + 1) * P, :])
                    return hTs

                def load_h(b):
                    h_s = hnpool.tile([P, KT, D], BF16, tag="h",
                                      name=f"h_s{b}")
                    nc.sync.dma_start(
                        h_s[:],
                        h_d.ap()[b].rearrange("(kt p) d -> p kt d", p=P))
                    return h_s

                next_hTs = None
                for b in range(BPC):
                    hTs = hT0 if b == 0 else next_hTs
                    h_s = None  # loaded mid-loop (deadline: our tail)
                    cov_row = rpool.tile([1, T], F32, tag="cov")
                    nc.gpsimd.dma_start(cov_row[:], cov_d.ap()[b:b + 1, :])

                    # rank-1 coverage tiles: build two ahead of the m-loop,
                    # then one per iteration, so the DVE never bursts and
                    # delays the ft adds
                    covWc = {}

                    def build_covWc(m):
                        cw = tpool.tile([P, T], BF16, tag=f"covWc{m}",
                                        bufs=2, name=f"covWc{b}_{m}")
                        nc.vector.tensor_scalar_mul(
                            cw[:], covbc_all[:, b, :], Wc_col[:, m:m + 1])
                        covWc[m] = cw

                    build_covWc(0)
                    build_covWc(1)
                    et_ps = etpool.tile([1, T], F32)
                    tanh_tiles = {}
                    for m in range(MT):
                        ft = ftpool.tile([P, T], F32, tag="ft",
                                         name=f"ft{b}_{m}")
                        if b == 0 and 1 <= m <= 2:
                            # keep the PE warm through batch-0's DMA dribble
                            for _ in range(4):
                                nc.tensor.matmul(
                                    ft[:, 0:NHS], zero_bf[:, 0:P],
                                    zero_bf[:], start=True, stop=True)
                        for kt in range(KT):
                            for n in range(NH):
                                nc.tensor.matmul(
                                    ft[:, n * NHS:(n + 1) * NHS],
                                    Wh_s[:, kt, m * P:(m + 1) * P],
                                    hTs[kt][:, n * NHS:(n + 1) * NHS],
                                    start=(kt == 0), stop=(kt == KT - 1))
                        if m + 2 < MT:
                            build_covWc(m + 2)
                        nc.vector.tensor_add(ft[:], ft[:], covWc.pop(m)[:])
                        tanh_t = tpool.tile([P, T], BF16, tag="tanh")
                        nc.scalar.activation(
                            tanh_t[:], ft[:], AF.Tanh,
                            bias=dec_T[:, m, b:b + 1])
                        tanh_tiles[m] = tanh_t

                        # previous batch's PE tail after our first m-tile
                        if m == 1 and b > 0:
                            emit_tail(b - 1, *state.pop(b - 1))
                        # DMA emission order = deadline order on one queue:
                        # next batch's hT before our h (used one tail later)
                        if m == 2 and b < BPC - 1:
                            next_hTs = load_hT(b + 1)
                        if m == 5:
                            h_s = load_h(b)

                        # e_t matmuls, two m-steps behind the FT matmuls
                        # (hides the DVE rank-1 add + tanh latency)
                        if m > 1:
                            prev = tanh_tiles.pop(m - 2)
                            for n in range(NH):
                                nc.tensor.matmul(
                                    et_ps[0:1, n * NHS:(n + 1) * NHS],
                                    V_col[:, m - 2:m - 1],
                                    prev[:, n * NHS:(n + 1) * NHS],
                                    start=(m == 2), stop=False)
                    for mm in (MT - 2, MT - 1):
                        last = tanh_tiles.pop(mm)
                        for n in range(NH):
                            nc.tensor.matmul(
                                et_ps[0:1, n * NHS:(n + 1) * NHS],
                                V_col[:, mm:mm + 1],
                                last[:, n * NHS:(n + 1) * NHS],
                                start=False, stop=(mm == MT - 1))

                    # softmax pieces on ScalarE/DVE (no PE dependency)
                    exp_row = rpool.tile([1, T], F32, tag="exp")
                    esum = rpool.tile([1, 1], F32, tag="esum")
                    nc.scalar.activation(
                        exp_row[:], et_ps[:], AF.Exp, accum_out=esum[:])
                    rsum = rpool.tile([1, 1], F32, tag="rsum")
                    nc.vector.reciprocal(rsum[:], esum[:])

                    a_row = rpool.tile([1, T], F32, tag="a")
                    nc.scalar.activation(
                        a_row[:], exp_row[:], AF.Copy, scale=rsum[:])
                    nc.sync.dma_start(a_d.ap()[b:b + 1, :], a_row[:])
                    covn_row = rpool.tile([1, T], F32, tag="covn")
                    nc.vector.tensor_add(
                        covn_row[:], a_row[:], cov_row[0:1, :])
                    nc.sync.dma_start(covn_d.ap()[b:b + 1, :], covn_row[:])

                    state[b] = (h_s, exp_row, rsum)

                emit_tail(BPC - 1, *state.pop(BPC - 1))

    nc.compile()
    return nc


def _get_nc():
    if "nc" not in _cache:
        _cache["nc"] = _build_nc()
    return _cache["nc"]


def prep_in_maps(h_i, s_t, coverage, W_h, W_s, b_s, W_c, V_w):
    import ml_dtypes

    bf16 = ml_dtypes.bfloat16
    h_i = np.ascontiguousarray(np.asarray(h_i, dtype=np.float32))
    s_t = np.asarray(s_t, dtype=np.float32)
    coverage = np.ascontiguousarray(np.asarray(coverage, dtype=np.float32))
    W_h = np.asarray(W_h, dtype=np.float32)
    W_s = np.asarray(W_s, dtype=np.float32)
    b_s = np.asarray(b_s, dtype=np.float32)
    W_c = np.asarray(W_c, dtype=np.float32)
    V_w = np.asarray(V_w, dtype=np.float32)

    h_bf = h_i.astype(bf16)
    hT_bf = np.ascontiguousarray(h_i.transpose(0, 2, 1)).astype(bf16)
    Wh_bf = np.ascontiguousarray(W_h.astype(bf16))
    covb = coverage.astype(bf16)
    Wc_bf = np.ascontiguousarray(W_c.astype(bf16))
    Vw_bf = np.ascontiguousarray(V_w.astype(bf16))
    dec = s_t @ W_s + b_s[None, :]  # (B, D) fp32, 0.1% of the FLOPs

    in_maps = []
    for c in range(NCORES):
        lo, hi = c * BPC, (c + 1) * BPC
        decT = np.ascontiguousarray(dec[lo:hi].T)
        in_maps.append({
            "hT": hT_bf[lo:hi],
            "h": h_bf[lo:hi],
            "Wh": Wh_bf,
            "decT": decT,
            "cov": coverage[lo:hi],
            "covb": covb[lo:hi],
            "Wc": Wc_bf,
            "Vw": Vw_bf,
        })
    return in_maps


def kernel(h_i, s_t, coverage, W_h, W_s, b_s, W_c, V_w):
    import time

    from concourse import bass_utils

    in_maps = prep_in_maps(h_i, s_t, coverage, W_h, W_s, b_s, W_c, V_w)
    nc = _get_nc()
    res = None
    for attempt in range(3):
        try:
            res = bass_utils.run_bass_kernel_spmd(
                nc, in_maps, core_ids=list(range(NCORES)))
            break
        except Exception:
            # transient NRT device errors recover on retry
            if attempt == 2:
                raise
            time.sleep(10)

    ctx = np.concatenate([res.results[c]["ctx"] for c in range(NCORES)], 0)
    a_t = np.concatenate([res.results[c]["a"] for c in range(NCORES)], 0)
    covn = np.concatenate([res.results[c]["covn"] for c in range(NCORES)], 0)
    return ctx, a_t, covn
